# revision 15
# baseline (speedup 1.0000x reference)
"""CRF Viterbi decode kernel for Trainium2 (Bass/Tile), 8-core data-parallel.

Problem: B=64, L=1024, D=1024, C=128 tags (126 classes + START/STOP).
  emis = x @ W + b                     [B,L,C]
  Viterbi forward scan over L steps (max-plus vector-matrix product per step
  with first-index argmax backpointers), then backtracking.

Sharding: data-parallel over batch, 8 examples per core; fc weights and the
128x128 transition matrix replicated.

Per-core design (BC=8 examples):
 - Emissions: x is split on host into bf16 hi/lo pairs (x = hi + lo to
   ~2^-16 relative); emis = xhi@Whi + xhi@Wlo + xlo@Whi + b accumulated in
   fp32 PSUM (max error ~1.6e-5, far below the ~1e-4 path-flip cliff).
   x tiles are transpose-DMA'd (xbar, 2-byte dtype) to [K,partitions] layout.
 - Forward scan, step l: PE broadcasts score rows into PSUM (ones-matmul) and
   accumulates T^T on top (transpose-mode matmul), giving
   ns[j,(b,i)] = scores[b,i] + T[i,j] in PSUM with zero DVE cost.
   DVE does one segmented reduce_max -> m[j,b]; backpointers via per-example
   scalar_tensor_tensor cand = (ns == m) * (128 - i) and a second reduce_max,
   which gives exact first-index argmax (jnp.argmax tie semantics).
   Masked steps: scores keep old value, bp forced to identity (select).
 - Backpointers are transposed (PE) and affine-cast (ACT: 128 - revbp ->
   uint16) into row-major [b, l, j] tables staged out to DRAM.
 - Backtracking: one gpsimd indirect_copy per step; example b's bp row lives
   on partition 16*b (one gpsimd core per example), the gathered tag is
   written into path_buf[:, l-1] which directly feeds the next step's index.
 - Output: path rows * prefix-mask, cast to int32.
"""

import numpy as np
import ml_dtypes

import concourse.bass as bass
import concourse.bacc as bacc
import concourse.tile as tile
import concourse.mybir as mybir

F32 = mybir.dt.float32
BF16 = mybir.dt.bfloat16
U8 = mybir.dt.uint8
U16 = mybir.dt.uint16
I32 = mybir.dt.int32

C = 128          # tags (incl START/STOP)
BC = 8           # examples per core
D = 1024
AX = mybir.AxisListType
OP = mybir.AluOpType
AF = mybir.ActivationFunctionType


def build_crf(L=1024, n_bufs_x=72):
    """Build the per-core Bass module. Returns nc."""
    nc = bacc.Bacc()

    # ---- DRAM I/O ----
    # x pre-transposed on host to [b, k, l] so device loads are contiguous
    xhi_d = nc.dram_tensor("xhi", [BC, D, L], BF16, kind="ExternalInput")
    xlo_d = nc.dram_tensor("xlo", [BC, D, L], BF16, kind="ExternalInput")
    whi_d = nc.dram_tensor("whi", [D, C], BF16, kind="ExternalInput")
    wlo_d = nc.dram_tensor("wlo", [D, C], BF16, kind="ExternalInput")
    bias_d = nc.dram_tensor("bias", [C, 1], F32, kind="ExternalInput")
    ta_d = nc.dram_tensor("ta", [C, C], F32, kind="ExternalInput")  # transitions[i,j]
    ident_d = nc.dram_tensor("ident", [C, C], F32, kind="ExternalInput")
    riota_d = nc.dram_tensor("riota", [C, C], F32, kind="ExternalInput")    # 128 - freeidx
    riotac_d = nc.dram_tensor("riotac", [C, BC], F32, kind="ExternalInput")  # 128 - partidx
    sel_d = nc.dram_tensor("sel", [BC, BC, C], F32, kind="ExternalInput")  # sel[k,b,j]=(k==b)
    mrep_d = nc.dram_tensor("mrep", [C, L, BC], U8, kind="ExternalInput")
    mrow_d = nc.dram_tensor("mrow", [BC, L], U16, kind="ExternalInput")
    out_d = nc.dram_tensor("paths", [BC, L], I32, kind="ExternalOutput")

    HL = min(128, L)   # x-DMA l-chunk
    SL = min(64, L)    # psum emission l-super
    CH = min(128, L)   # chase phase l-chunk

    with tile.TileContext(nc) as tc:
        import contextlib
        with contextlib.ExitStack() as top:
            consts = top.enter_context(tc.tile_pool(name="consts", bufs=1))
            ta = consts.tile([C, C], F32)
            ident = consts.tile([C, C], F32)
            riota = consts.tile([C, C], F32)
            riotac = consts.tile([C, BC], F32)
            sel = consts.tile([BC, BC, C], F32)
            bias = consts.tile([C, 1], F32)
            whi = consts.tile([C, D // C, C], BF16)
            wlo = consts.tile([C, D // C, C], BF16)
            mrep = consts.tile([C, L, BC], U8)
            mrow = consts.tile([BC, L], U16)
            emis = consts.tile([C, L, BC], F32)
            # path2[p, 2l:2l+2] holds (tag_l, junk) pairs: gathers are 2 elems
            # wide (4B) because 1-elem dst tensors fail the ISA mem4d check
            path2 = consts.tile([C, 2 * L], U16)
            nc.vector.memset(path2, 0)

            nc.sync.dma_start(out=ta, in_=ta_d[:, :])
            nc.sync.dma_start(out=ident, in_=ident_d[:, :])
            nc.sync.dma_start(out=riota, in_=riota_d[:, :])
            nc.sync.dma_start(out=riotac, in_=riotac_d[:, :])
            nc.sync.dma_start(out=sel, in_=sel_d[:, :, :])
            nc.sync.dma_start(out=bias, in_=bias_d[:, :])
            nc.sync.dma_start(out=mrep, in_=mrep_d[:, :, :])
            nc.sync.dma_start(out=mrow, in_=mrow_d[:, :])
            # W chunks: whi[:, k, :] = W[k*128:(k+1)*128, :]
            nc.sync.dma_start(
                out=whi, in_=whi_d.rearrange("(k p) c -> p k c", p=C))
            nc.sync.dma_start(
                out=wlo, in_=wlo_d.rearrange("(k p) c -> p k c", p=C))

            dram = top.enter_context(
                tc.tile_pool(name="dram", bufs=1, space="DRAM"))
            chase_dram = dram.tile([BC, L, C], U16)

            # ================= Phase 1: emissions =================
            with contextlib.ExitStack() as ectx:
                xpool = ectx.enter_context(
                    tc.tile_pool(name="xt", bufs=n_bufs_x))
                epsum = ectx.enter_context(
                    tc.tile_pool(name="epsum", bufs=2, space="PSUM"))
                for h0 in range(0, L, HL):
                    xh_t = {}
                    xl_t = {}
                    for k in range(D // C):
                        for b in range(BC):
                            th = xpool.tile([C, HL], BF16, tag="xh")
                            tl = xpool.tile([C, HL], BF16, tag="xl")
                            nc.sync.dma_start(
                                out=th, in_=xhi_d[b, k * C:(k + 1) * C, h0:h0 + HL])
                            nc.sync.dma_start(
                                out=tl, in_=xlo_d[b, k * C:(k + 1) * C, h0:h0 + HL])
                            xh_t[k, b] = th
                            xl_t[k, b] = tl
                    for s0 in range(0, HL, SL):
                        ps = epsum.tile([C, BC * SL], F32)
                        mms = []
                        n_mm = (D // C) * 3 * BC
                        for k in range(D // C):
                            for wt, xt in ((whi, xh_t), (wlo, xh_t), (whi, xl_t)):
                                for b in range(BC):
                                    i = len(mms)
                                    mm = nc.tensor.matmul(
                                        ps[:, b * SL:(b + 1) * SL],
                                        lhsT=wt[:, k, :],
                                        rhs=xt[k, b][:, s0:s0 + SL],
                                        start=(i == 0),
                                        stop=(i == n_mm - 1),
                                        skip_group_check=True,
                                    )
                                    if mms:
                                        bass._add_dep_helper(
                                            mm.ins, mms[-1].ins, sync=False,
                                            reason="psum group order")
                                    mms.append(mm)
                        # bias add + relayout (b,l) -> (l,b)
                        dst = emis[:, h0 + s0:h0 + s0 + SL, :].rearrange(
                            "p l b -> p b l")
                        nc.scalar.activation(
                            out=dst, in_=ps.rearrange("p (b l) -> p b l", b=BC),
                            func=AF.Identity, bias=bias, scale=1.0)

            # ================= Phase 2: forward scan =================
            with contextlib.ExitStack() as sctx:
                spool = sctx.enter_context(tc.tile_pool(name="scan", bufs=3))
                nspsum = sctx.enter_context(
                    tc.tile_pool(name="ns", bufs=2, space="PSUM"))
                rpsum = sctx.enter_context(
                    tc.tile_pool(name="rows", bufs=1, space="PSUM"))
                bpsum = sctx.enter_context(
                    tc.tile_pool(name="bpt", bufs=1, space="PSUM"))
                stg = sctx.enter_context(tc.tile_pool(name="stg", bufs=2))

                scores = spool.tile([C, BC], F32, tag="scores")
                nc.vector.memset(scores, 0.0)

                staging = None
                for l in range(L):
                    ns = nspsum.tile([C, BC * C], F32)
                    if l > 0:
                        # score rows [b, i] via PE transpose + ACT copy
                        rows_ps = rpsum.tile([BC, C], F32)
                        nc.tensor.transpose(rows_ps, scores, ident)
                        rows = spool.tile([BC, C], F32, tag="rows")
                        nc.scalar.activation(
                            out=rows, in_=rows_ps, func=AF.Copy)
                    # per 2KB bank (4 regions): one start, chain, one stop
                    for bank in range(2):
                        prev = None
                        ops = []
                        bs = range(bank * 4, bank * 4 + 4)
                        if l > 0:
                            for b in bs:
                                ops.append(dict(
                                    out=ns[:, b * C:(b + 1) * C],
                                    lhsT=sel[:, b, :], rhs=rows))
                        for b in bs:
                            ops.append(dict(
                                out=ns[:, b * C:(b + 1) * C],
                                lhsT=ta, rhs=ident, is_transpose=True))
                        for i, kw in enumerate(ops):
                            out_ap = kw.pop("out")
                            mm = nc.tensor.matmul(
                                out_ap, start=(i == 0), stop=(i == len(ops) - 1),
                                skip_group_check=True, **kw)
                            if prev is not None:
                                bass._add_dep_helper(
                                    mm.ins, prev.ins, sync=False,
                                    reason="psum group order")
                            prev = mm

                    ns3 = ns.rearrange("p (b i) -> p b i", b=BC)
                    m = spool.tile([C, BC], F32, tag="m")
                    nc.vector.reduce_max(out=m, in_=ns3, axis=AX.X)

                    mask_l = mrep[:, l, :]
                    # new scores = select(mask, m + e_l, old)
                    newsc = spool.tile([C, BC], F32, tag="new")
                    nc.vector.tensor_add(newsc, m, emis[:, l, :])
                    sc2 = spool.tile([C, BC], F32, tag="scores")
                    nc.vector.tensor_copy(sc2, scores)
                    nc.vector.copy_predicated(sc2, mask_l, newsc)

                    # backpointers: cand = (ns == m) * (128 - i); max -> 128-argmax
                    cand = spool.tile([C, BC * C], F32, tag="cand")
                    for b in range(BC):
                        nc.vector.scalar_tensor_tensor(
                            out=cand[:, b * C:(b + 1) * C],
                            in0=ns[:, b * C:(b + 1) * C],
                            scalar=m[:, b:b + 1], in1=riota,
                            op0=OP.is_equal, op1=OP.mult)
                    rbp_raw = spool.tile([C, BC], F32, tag="rbp_raw")
                    nc.vector.reduce_max(
                        out=rbp_raw,
                        in_=cand.rearrange("p (b i) -> p b i", b=BC), axis=AX.X)
                    rbp = spool.tile([C, BC], F32, tag="rbp")
                    nc.vector.tensor_copy(rbp, riotac)
                    nc.vector.copy_predicated(rbp, mask_l, rbp_raw)

                    # stage bp rows: transpose + (128 - revbp) as uint16
                    bpt_ps = bpsum.tile([BC, C], F32)
                    nc.tensor.transpose(bpt_ps, rbp, ident)
                    if l % 16 == 0:
                        staging = stg.tile([BC, 16 * C], U16, tag="stage")
                    nc.scalar.activation(
                        out=staging[:, (l % 16) * C:(l % 16 + 1) * C],
                        in_=bpt_ps, func=AF.Copy, bias=128.0, scale=-1.0)
                    if l % 16 == 15:
                        nc.sync.dma_start(
                            out=chase_dram[:, l - 15:l + 1, :],
                            in_=staging.rearrange("p (l j) -> p l j", l=16))
                    scores = sc2

                # ---- finale: last_tag = argmax(scores + T[:, STOP]) ----
                fin = spool.tile([C, BC], F32, tag="fin")
                nc.vector.tensor_scalar_add(fin, scores, ta[:, C - 1:C])
                fin_ps = rpsum.tile([BC, C], F32)
                nc.tensor.transpose(fin_ps, fin, ident)
                finr = spool.tile([BC, C], F32, tag="finr")
                nc.scalar.activation(out=finr, in_=fin_ps, func=AF.Copy)
                mf = spool.tile([BC, 1], F32, tag="mf")
                nc.vector.reduce_max(out=mf, in_=finr, axis=AX.X)
                candf = spool.tile([BC, C], F32, tag="candf")
                nc.vector.scalar_tensor_tensor(
                    out=candf, in0=finr, scalar=mf, in1=riota[0:BC, :],
                    op0=OP.is_equal, op1=OP.mult)
                rlast = spool.tile([BC, 1], F32, tag="rlast")
                nc.vector.reduce_max(out=rlast, in_=candf, axis=AX.X)
                last16 = spool.tile([BC, 1], U16, tag="last16")
                nc.scalar.activation(
                    out=last16, in_=rlast, func=AF.Copy, bias=128.0, scale=-1.0)
                # scatter to path2[16b, 2(L-1)]
                nc.sync.dma_start(
                    out=path2.rearrange(
                        "(b g) f -> b g f", g=16)[:BC, 0, 2 * (L - 1):2 * L - 1],
                    in_=last16)

            # ================= Phase 3: backtracking chase =================
            tc.strict_bb_all_engine_barrier()
            with contextlib.ExitStack() as cctx:
                ph = cctx.enter_context(tc.tile_pool(name="phase", bufs=2))
                for p0 in range(L - CH, -1, -CH):
                    pb = ph.tile([C, CH * C + 2], U16, tag="phase")
                    nc.vector.memset(pb, 0)
                    nc.sync.dma_start(
                        out=pb.rearrange(
                            "p f -> p f")[:, 0:CH * C].rearrange(
                            "(b g) f -> b g f", g=16)[:BC, 0, :],
                        in_=chase_dram[:, p0:p0 + CH, :].rearrange(
                            "b l j -> b (l j)"))
                    for l in range(p0 + CH - 1, max(p0 - 1, 0), -1):
                        o = l - p0
                        nc.gpsimd.indirect_copy(
                            out=path2[:, 2 * (l - 1):2 * l].rearrange(
                                "p (a t) -> p a t", t=2),
                            data=pb[:, o * C:o * C + 130].rearrange(
                                "p (a t) -> p a t", t=2),
                            idxs=path2[:, 2 * l:2 * l + 1],
                            i_know_ap_gather_is_preferred=True)

                # ---- output: rows * prefix-mask, cast to int32 ----
                orow = ph.tile([BC, L], U16, tag="orow")
                nc.sync.dma_start(
                    out=orow,
                    in_=path2.rearrange(
                        "(b g) (l t) -> b g l t", g=16, t=2)[:BC, 0, :, 0])
                omask = ph.tile([BC, L], U16, tag="omask")
                nc.vector.tensor_mul(omask, orow, mrow)
                oi32 = ph.tile([BC, L], I32, tag="oi32")
                nc.vector.tensor_copy(oi32, omask)
                nc.sync.dma_start(out=out_d[:, :], in_=oi32)

    nc.finalize()
    return nc


def make_core_inputs(x, mask, W, b, T, L):
    """Host-side prep of one core's input dict (x: [BC, L, D] fp32)."""
    bf = ml_dtypes.bfloat16
    xhi = x.astype(bf)
    xlo = (x - xhi.astype(np.float32)).astype(bf)
    xhi = np.ascontiguousarray(xhi.transpose(0, 2, 1))
    xlo = np.ascontiguousarray(xlo.transpose(0, 2, 1))
    whi = W.astype(bf)
    wlo = (W - whi.astype(np.float32)).astype(bf)
    m8 = mask.astype(np.uint8)          # [BC, L]
    mrep = np.broadcast_to(m8.T[None, :, :], (C, L, BC)).copy()  # [C, L, BC]
    mrow = mask.astype(np.uint16)        # [BC, L]
    riota = np.broadcast_to(
        (C - np.arange(C, dtype=np.float32))[None, :], (C, C)).copy()
    riotac = np.broadcast_to(
        (C - np.arange(C, dtype=np.float32))[:, None], (C, BC)).copy()
    return {
        "xhi": xhi, "xlo": xlo, "whi": whi, "wlo": wlo,
        "bias": b.reshape(C, 1).astype(np.float32),
        "ta": T.astype(np.float32),
        "ident": np.eye(C, dtype=np.float32),
        "sel": np.broadcast_to(
            np.eye(BC, dtype=np.float32)[:, :, None], (BC, BC, C)).copy(),
        "riota": riota.astype(np.float32),
        "riotac": riotac.astype(np.float32),
        "mrep": mrep, "mrow": mrow,
    }


_NC_CACHE = {}
LAST_RESULT = None


def kernel(x, mask, y, W, b, transitions):
    """Full-input entry: shard over 8 cores, run, gather."""
    from concourse.bass_utils import run_bass_kernel_spmd

    x = np.asarray(x, dtype=np.float32)
    mask = np.asarray(mask)
    W = np.asarray(W, dtype=np.float32)
    b = np.asarray(b, dtype=np.float32)
    T = np.asarray(transitions, dtype=np.float32)
    B, L, _ = x.shape
    n_cores = 8
    bc = B // n_cores

    if L not in _NC_CACHE:
        _NC_CACHE[L] = build_crf(L=L)
    nc = _NC_CACHE[L]

    in_maps = []
    for c in range(n_cores):
        sl = slice(c * bc, (c + 1) * bc)
        in_maps.append(make_core_inputs(x[sl], mask[sl], W, b, T, L))

    res = run_bass_kernel_spmd(nc, in_maps, core_ids=list(range(n_cores)))
    global LAST_RESULT
    LAST_RESULT = res
    out = np.concatenate([r["paths"] for r in res.results], axis=0)
    return out.astype(np.int32)
